# revision 7
# baseline (speedup 1.0000x reference)
"""Tied-row (MSA) attention, sharded over 8 TRN2 NeuronCores.

Reference computation (b=1, r=128 MSA rows, n=512, 8 heads x 64):
    q, k, v = x @ Wq, x @ Wk, x @ Wv          per-row projections
    dots[h,i,j] = sum_{r,d} q[r,h,i,d] k[r,h,j,d] * scale / sqrt(num_rows)
    attn = softmax_j(dots)                     shared across rows
    out[r,i] = (sum_j attn[h,i,j] v[r,h,j,d]) @ Wo + bo

Sharding: MSA-row axis r split 16-per-core; partial logits are AllReduced
(one bf16 AllReduce per head-pair, pipelined behind later pairs' compute).

Every matmul runs at full 128-wide PE contraction by packing MSA ROW-PAIRS
into the partition axis:
  - dots: qP/kP tiles hold (row-pair, head) data as [rho*64+d, token]; one
    K=128 matmul reduces two rows at once (the tied-row r-reduction makes
    the cross-row sum exactly what we want).  256 matmuls instead of 512.
  - attn@v: vP tiles [j, h, rho*64+d] give a [128,128] stationary per
    (head, row-pair); the row-tied attention tile is the shared moving
    side.  Output [(rho,d), i] psum halves route to per-row oT tiles
    (cross-partition-offset evictions).  256 matmuls instead of 512.
  - x transposes on the PE in fp32 (2 cyc/row); the fp32->bf16 cast rides
    the psum eviction, removing the DVE cast pass entirely.
  - softmax: Z[i] from a [128,1]-ones matmul, 1/Z via the ACT Reciprocal
    LUT on the [1,512] row, broadcast across partitions with a K=1 matmul.
"""

import numpy as np

import concourse.bacc as bacc
import concourse.bass as bass
import concourse.mybir as mybir
import concourse.tile as tile
from concourse import bass_utils
from concourse.masks import make_identity

CORES = 8
R = 16          # MSA rows per core
RP = R // 2     # row pairs per core
N = 512         # sequence length
DIM = 256       # model dim
H = 8           # heads
D = 64          # head dim
HD = H * D      # 512
RN = R * N      # 8192 token-rows per core

F32 = mybir.dt.float32
BF16 = mybir.dt.bfloat16
AF = mybir.ActivationFunctionType

RG = [list(range(CORES))]


def build_nc(scale: float):
    nc = bacc.Bacc(None, target_bir_lowering=False, debug=False)

    x_ext = nc.declare_dram_parameter("x", [RN, DIM], F32, isOutput=False)
    wq_ext = nc.declare_dram_parameter("wq", [DIM, HD], F32, isOutput=False)
    wk_ext = nc.declare_dram_parameter("wk", [DIM, HD], F32, isOutput=False)
    wv_ext = nc.declare_dram_parameter("wv", [DIM, HD], F32, isOutput=False)
    wo_ext = nc.declare_dram_parameter("wo", [HD, DIM], F32, isOutput=False)
    out_ext = nc.declare_dram_parameter("out", [RN, DIM], F32, isOutput=True)

    # alternate PSUM->SBUF evictions between DVE and ScalarE
    _cp = [0]

    def cp(out, in_):
        if _cp[0] % 2 == 0:
            nc.vector.tensor_copy(out, in_)
        else:
            nc.scalar.copy(out, in_)
        _cp[0] += 1

    def dma(out, in_):
        nc.sync.dma_start(out=out, in_=in_)

    with tile.TileContext(nc) as tc:
        # ---- DRAM bounce buffers: one AllReduce per head-pair ----
        dram = tc.alloc_tile_pool(name="dram", bufs=1, space="DRAM")
        ar_in = [dram.tile([2 * N, N], BF16, tag=f"ar_in{hp}", name=f"ar_in{hp}") for hp in range(4)]
        wu_in = dram.tile([128, 8], BF16, tag="wu_in", name="wu_in")
        wu_out = dram.tile([128, 8], BF16, tag="wu_out", name="wu_out", addr_space="Shared")
        ar_out = [
            dram.tile([2 * N, N], BF16, tag=f"ar_out{hp}", name=f"ar_out{hp}", addr_space="Shared")
            for hp in range(4)
        ]

        # ---- SBUF pools, stacked so releases stay LIFO ----
        consts = tc.alloc_tile_pool(name="consts", bufs=1)
        vP_pool = tc.alloc_tile_pool(name="vP", bufs=RP * 4)       # 32 x 2KB
        et_pool = tc.alloc_tile_pool(name="et", bufs=8)            # 8KB
        attn_pool = tc.alloc_tile_pool(name="attn", bufs=16)       # 16KB
        rz_pool = tc.alloc_tile_pool(name="rz", bufs=2)
        rzbc_pool = tc.alloc_tile_pool(name="rzbc", bufs=4)
        zt_pool = tc.alloc_tile_pool(name="zt", bufs=2)            # 8KB
        xT_pool = tc.alloc_tile_pool(name="xT", bufs=1)            # 32KB
        xrow_pool = tc.alloc_tile_pool(name="xrow", bufs=4)        # 16KB
        wstage = tc.alloc_tile_pool(name="wstage", bufs=2)         # 8KB

        # constants
        wq_sb = consts.tile([128, 2, HD], BF16, tag="wq")
        wk_sb = consts.tile([128, 2, HD], BF16, tag="wk")
        wv_sb = consts.tile([128, 2, HD], BF16, tag="wv")
        wo_sb = consts.tile([128, 4, DIM], BF16, tag="wo")
        idf = consts.tile([128, 128], F32, tag="idf")
        ones_bf = consts.tile([128, 128], BF16, tag="ones_bf")
        onesf = consts.tile([128, 128], F32, tag="onesf")

        xT = xT_pool.tile([128, 2, RN], BF16, tag="xT")

        # first x chunk prefetch (4 row-blocks of 128 per DMA)
        first_xr = xrow_pool.tile([128, 4, DIM], F32, tag="xr")
        nc.sync.dma_start(
            out=first_xr[:],
            in_=x_ext[0:512, :].rearrange("(j p) d -> p j d", p=128),
        )

        for weng, wext, wsb in ((nc.sync, wq_ext, wq_sb), (nc.scalar, wk_ext, wk_sb)):
            wf = wstage.tile([128, 2, HD], F32, tag="wf")
            weng.dma_start(
                out=wf[:], in_=wext[:, :].rearrange("(k p) n -> p k n", p=128)
            )
            nc.any.tensor_copy(wsb[:], wf[:])
        make_identity(nc, idf[:])
        nc.vector.memset(onesf[:], 1.0)
        nc.vector.tensor_copy(ones_bf[:], onesf[:])

        # warm up ncfw so the first real AllReduce skips the cold-start lag
        nc.sync.dma_start(out=wu_in[:, :], in_=ones_bf[:, 0:8])
        nc.gpsimd.collective_compute(
            "AllReduce",
            mybir.AluOpType.add,
            replica_groups=RG,
            ins=[wu_in[:, :].opt()],
            outs=[wu_out[:, :].opt()],
        )

        # ---- x load + fp32 PE transpose -> xT bf16 [dim(2x128), rn] ----
        xp_psum = tc.alloc_tile_pool(name="xp_psum", bufs=2, space="PSUM")
        for c4 in range(RN // N):
            if c4 == 0:
                xr = first_xr
            else:
                xr = xrow_pool.tile([128, 4, DIM], F32, tag="xr")
                eng = nc.scalar if (c4 % 2 == 1) else nc.sync
                eng.dma_start(
                    out=xr[:],
                    in_=x_ext[c4 * 512:(c4 + 1) * 512, :].rearrange(
                        "(j p) d -> p j d", p=128
                    ),
                )
            for kc in range(2):
                pt = xp_psum.tile([128, N], F32, tag="xp")
                for j in range(4):
                    nc.tensor.transpose(
                        pt[:, j * 128:(j + 1) * 128],
                        xr[:, j, kc * 128:(kc + 1) * 128],
                        idf[:],
                    )
                cp(xT[:, kc, c4 * N:(c4 + 1) * N], pt[:])
        xp_psum.release()

        # late weight staging (wv used ~2/3 in, wo in the last quarter)
        wvf = wstage.tile([128, 2, HD], F32, tag="wf")
        nc.sync.dma_start(
            out=wvf[:], in_=wv_ext[:, :].rearrange("(k p) n -> p k n", p=128)
        )
        nc.any.tensor_copy(wv_sb[:], wvf[:])
        wof = wstage.tile([128, 4, DIM], F32, tag="wf")
        nc.sync.dma_start(
            out=wof[:], in_=wo_ext[:, :].rearrange("(k p) n -> p k n", p=128)
        )
        nc.any.tensor_copy(wo_sb[:], wof[:])
        wstage.release()
        xrow_pool.release()

        proj_psum = tc.alloc_tile_pool(name="proj_psum", bufs=2, space="PSUM")
        dots_psum = tc.alloc_tile_pool(name="dots_psum", bufs=4, space="PSUM")
        z_psum = tc.alloc_tile_pool(name="z_psum", bufs=1, space="PSUM")

        stage_pool = tc.alloc_tile_pool(name="stage", bufs=2)      # 8KB
        qkP_pool = tc.alloc_tile_pool(name="qkP", bufs=40)         # 40KB

        attn = {}

        def softmax(hp, zpool, wait_ms):
            """exp + Z + 1/Z broadcast + normalize for AllReduce #hp."""
            with tc.tile_wait_until(wait_ms):
                for m in range(2):
                    h = 2 * hp + m
                    zt = zt_pool.tile([128, 4, N], BF16, tag="zt")
                    dma(
                        zt[:],
                        ar_out[hp][m * N:(m + 1) * N, :].rearrange(
                            "(jc p) n -> p jc n", p=128
                        ),
                    )
                    ets = []
                    for jc in range(4):
                        et = et_pool.tile([128, N], BF16, tag="et")
                        nc.scalar.activation(et[:], zt[:, jc, :], AF.Exp, scale=scale)
                        ets.append(et)
                    zp = zpool.tile([1, N], F32, tag="zp")
                    for jc in range(4):
                        nc.tensor.matmul(
                            zp[:],
                            ones_bf[:, 0:1],
                            ets[jc][:],
                            start=(jc == 0),
                            stop=(jc == 3),
                        )
                    lnz = rz_pool.tile([1, N], F32, tag="lnz")
                    nc.scalar.activation(lnz[:], zp[:], AF.Ln)
                    rz = rz_pool.tile([1, N], BF16, tag="rz")
                    with nc.allow_low_precision(reason="1/Z scale fine in bf16"):
                        nc.scalar.activation(rz[:], lnz[:], AF.Exp, scale=-1.0)
                    bp = zpool.tile([128, N], F32, tag="bp")
                    nc.tensor.matmul(
                        bp[:], ones_bf[0:1, :], rz[:], start=True, stop=True
                    )
                    rb = rzbc_pool.tile([128, N], BF16, tag="rzbc")
                    cp(rb[:], bp[:])
                    for jc in range(4):
                        at = attn_pool.tile([128, N], BF16, tag="attn")
                        nc.vector.tensor_mul(at[:], ets[jc][:], rb[:])
                        attn[(h, jc)] = at

        # ---- per head-pair: project q,k (row-pair packed), dots, AllReduce
        qP = {}
        kP = {}
        for hp in range(4):
            if hp == 3:
                softmax(0, z_psum, 0.105)
            for rr in range(R):
                rrp, rho = rr >> 1, rr & 1
                for wsb, pk in ((wq_sb, qP), (wk_sb, kP)):
                    ps = proj_psum.tile([128, N], F32, tag="proj")
                    for kc in range(2):
                        nc.tensor.matmul(
                            ps[:],
                            wsb[:, kc, hp * 128:(hp + 1) * 128],
                            xT[:, kc, rr * N:(rr + 1) * N],
                            start=(kc == 0),
                            stop=(kc == 1),
                        )
                    if rho == 0:
                        pk[(2 * hp, rrp)] = qkP_pool.tile([128, N], BF16, tag="pk", name="pk_e")
                        pk[(2 * hp + 1, rrp)] = qkP_pool.tile([128, N], BF16, tag="pk", name="pk_o")
                    cp(pk[(2 * hp, rrp)][rho * 64:(rho + 1) * 64, :], ps[0:64, :])
                    cp(pk[(2 * hp + 1, rrp)][rho * 64:(rho + 1) * 64, :], ps[64:128, :])

            # dots: K=128 over (row-pair, d); rrp-major across 4 jc banks
            for m in range(2):
                h = 2 * hp + m
                st = stage_pool.tile([128, 4, N], BF16, tag="dstage")
                dps = [dots_psum.tile([128, N], F32, tag="dots", name=f"dots{jj}") for jj in range(4)]
                for rrp in range(RP):
                    for jc in range(4):
                        nc.tensor.matmul(
                            dps[jc][:],
                            kP[(h, rrp)][:, jc * 128:(jc + 1) * 128],
                            qP[(h, rrp)][:],
                            start=(rrp == 0),
                            stop=(rrp == RP - 1),
                            skip_group_check=True,
                        )
                for jc in range(4):
                    cp(st[:, jc, :], dps[jc][:])
                dma(
                    ar_in[hp][m * N:(m + 1) * N, :].rearrange(
                        "(jc p) n -> p jc n", p=128
                    ),
                    st[:],
                )

            nc.gpsimd.collective_compute(
                "AllReduce",
                mybir.AluOpType.add,
                replica_groups=RG,
                ins=[ar_in[hp][:, :].opt()],
                outs=[ar_out[hp][:, :].opt()],
            )

        # ---- v projection (overlaps the AllReduces; reads xT) ----
        vP = {}
        for rr in range(R):
            rrp, rho = rr >> 1, rr & 1
            if rr == 2:
                softmax(1, z_psum, 0.135)
            if rr == 9:
                softmax(2, z_psum, 0.165)
            for jt in range(4):
                ps = proj_psum.tile([128, N], F32, tag="proj")
                for kc in range(2):
                    nc.tensor.matmul(
                        ps[:],
                        xT[:, kc, rr * N + jt * 128:rr * N + jt * 128 + 128],
                        wv_sb[:, kc, :],
                        start=(kc == 0),
                        stop=(kc == 1),
                    )
                if rho == 0:
                    vP[(rrp, jt)] = vP_pool.tile([128, H, 128], BF16, tag="vP", name="vPt")
                cp(
                    vP[(rrp, jt)][:, :, rho * 64:(rho + 1) * 64],
                    ps[:].rearrange("p (h d) -> p h d", d=64),
                )

        z_psum.release()
        dots_psum.release()
        proj_psum.release()
        qkP_pool.release()
        stage_pool.release()
        xT_pool.release()

        # ---- attn^T @ v -> per-row oT, then out @ Wo ----
        oT_pool = tc.alloc_tile_pool(name="oT", bufs=R * 4)        # 64KB
        fst_pool = tc.alloc_tile_pool(name="fst", bufs=3)
        av_psum = tc.alloc_tile_pool(name="av_psum", bufs=3, space="PSUM")
        fin_psum = tc.alloc_tile_pool(name="fin_psum", bufs=2, space="PSUM")
        z2_psum = tc.alloc_tile_pool(name="z2_psum", bufs=1, space="PSUM")

        _oq = [0]
        oT = {}
        for hp in range(4):
            if hp == 0:
                softmax(3, z2_psum, 0.195)
            for rrp in range(RP):
                for m in range(2):
                    h = 2 * hp + m
                    ap_ = av_psum.tile([128, N], F32, tag="av")
                    for jt in range(4):
                        nc.tensor.matmul(
                            ap_[:],
                            vP[(rrp, jt)][:, h, :],
                            attn[(h, jt)][:],
                            start=(jt == 0),
                            stop=(jt == 3),
                        )
                    for rho in range(2):
                        r = 2 * rrp + rho
                        if (r, hp) not in oT:
                            oT[(r, hp)] = oT_pool.tile([128, N], BF16, tag="oT", name="oTt")
                        cp(
                            oT[(r, hp)][m * 64:(m + 1) * 64, :],
                            ap_[rho * 64:(rho + 1) * 64, :],
                        )
                if hp == 3:
                    # output projection for the two rows of this pair
                    for rho in range(2):
                        r = 2 * rrp + rho
                        fst = fst_pool.tile([128, 4, DIM], F32, tag="fst")
                        for icp in range(2):
                            psf = fin_psum.tile([128, 2, DIM], F32, tag="fin")
                            for ici in range(2):
                                ic = 2 * icp + ici
                                for kc in range(4):
                                    nc.tensor.matmul(
                                        psf[:, ici, :],
                                        oT[(r, kc)][:, ic * 128:(ic + 1) * 128],
                                        wo_sb[:, kc, :],
                                        start=(kc == 0),
                                        stop=(kc == 3),
                                    )
                            cp(fst[:, 2 * icp:2 * icp + 2, :], psf[:])
                        eng = nc.gpsimd if _oq[0] % 2 == 0 else nc.sync
                        eng.dma_start(
                            out=out_ext[r * N:(r + 1) * N, :].rearrange(
                                "(ic p) d -> p ic d", p=128
                            ),
                            in_=fst[:],
                        )
                        _oq[0] += 1

        z2_psum.release()
        fin_psum.release()
        av_psum.release()
        fst_pool.release()
        oT_pool.release()
        zt_pool.release()
        rzbc_pool.release()
        rz_pool.release()
        attn_pool.release()
        et_pool.release()
        vP_pool.release()
        consts.release()
        dram.release()

    if not nc.is_finalized():
        nc.finalize()
    return nc


_cache = {}


def _get_nc(scale: float):
    key = round(float(scale), 12)
    if key not in _cache:
        _cache[key] = build_nc(float(scale))
    return _cache[key]


def make_in_maps(x, Wq, Wkv, Wo):
    x = np.ascontiguousarray(np.asarray(x, dtype=np.float32)).reshape(CORES, RN, DIM)
    Wq = np.ascontiguousarray(np.asarray(Wq, dtype=np.float32))
    Wkv = np.asarray(Wkv, dtype=np.float32)
    Wk = np.ascontiguousarray(Wkv[:, :HD])
    Wv = np.ascontiguousarray(Wkv[:, HD:])
    Wo = np.ascontiguousarray(np.asarray(Wo, dtype=np.float32))
    return [
        {"x": x[c], "wq": Wq, "wk": Wk, "wv": Wv, "wo": Wo} for c in range(CORES)
    ]


def kernel(x, Wq, Wkv, Wo, bo, mask, tie_attn_dim):
    x = np.asarray(x)
    br, n, dim = x.shape
    r = int(tie_attn_dim)
    assert (br, n, dim) == (128, 512, 256) and r == 128, "kernel hardcodes shapes"
    mask = np.asarray(mask)
    assert mask.all(), "kernel assumes an all-valid mask"
    num_rows = float(mask.reshape(1, r, n).any(axis=-1).sum(axis=-1)[0])
    scale = (D ** -0.5) * (num_rows ** -0.5)

    nc = _get_nc(scale)
    in_maps = make_in_maps(x, Wq, Wkv, Wo)
    res = bass_utils.run_bass_kernel_spmd(nc, in_maps, core_ids=list(range(CORES)))
    out = np.concatenate([m["out"] for m in res.results], axis=0)
    out = out.reshape(br, n, dim)
    bo = np.asarray(bo, dtype=np.float32)
    if bo.any():
        out = out + bo
    return np.ascontiguousarray(out.astype(np.float32))


# revision 8
# speedup vs baseline: 1.1075x; 1.1075x over previous
"""Tied-row (MSA) attention, sharded over 8 TRN2 NeuronCores.

Reference computation (b=1, r=128 MSA rows, n=512, 8 heads x 64):
    q, k, v = x @ Wq, x @ Wk, x @ Wv          per-row projections
    dots[h,i,j] = sum_{r,d} q[r,h,i,d] k[r,h,j,d] * scale / sqrt(num_rows)
    attn = softmax_j(dots)                     shared across rows
    out[r,i] = (sum_j attn[h,i,j] v[r,h,j,d]) @ Wo + bo

Sharding: MSA-row axis r split 16-per-core; partial logits are AllReduced
(one bf16 AllReduce per head-pair, pipelined behind later pairs' compute).

Every matmul runs at full 128-wide PE contraction by packing MSA ROW-PAIRS
into the partition axis:
  - dots: qP/kP tiles hold (row-pair, head) data as [rho*64+d, token]; one
    K=128 matmul reduces two rows at once (the tied-row r-reduction makes
    the cross-row sum exactly what we want).  256 matmuls instead of 512.
  - attn@v: vP tiles [j, h, rho*64+d] give a [128,128] stationary per
    (head, row-pair); the row-tied attention tile is the shared moving
    side.  Output [(rho,d), i] psum halves route to per-row oT tiles
    (cross-partition-offset evictions).  256 matmuls instead of 512.
  - x transposes on the PE in fp32 (2 cyc/row); the fp32->bf16 cast rides
    the psum eviction, removing the DVE cast pass entirely.
  - softmax: Z[i] from a [128,1]-ones matmul, 1/Z via the ACT Reciprocal
    LUT on the [1,512] row, broadcast across partitions with a K=1 matmul.
"""

import numpy as np

import concourse.bacc as bacc
import concourse.bass as bass
import concourse.mybir as mybir
import concourse.tile as tile
from concourse import bass_utils
from concourse.masks import make_identity

CORES = 8
R = 16          # MSA rows per core
RP = R // 2     # row pairs per core
N = 512         # sequence length
DIM = 256       # model dim
H = 8           # heads
D = 64          # head dim
HD = H * D      # 512
RN = R * N      # 8192 token-rows per core

F32 = mybir.dt.float32
BF16 = mybir.dt.bfloat16
AF = mybir.ActivationFunctionType

RG = [list(range(CORES))]


def build_nc(scale: float):
    nc = bacc.Bacc(None, target_bir_lowering=False, debug=False)

    x_ext = nc.declare_dram_parameter("x", [RN, DIM], F32, isOutput=False)
    wq_ext = nc.declare_dram_parameter("wq", [DIM, HD], F32, isOutput=False)
    wk_ext = nc.declare_dram_parameter("wk", [DIM, HD], F32, isOutput=False)
    wv_ext = nc.declare_dram_parameter("wv", [DIM, HD], F32, isOutput=False)
    wo_ext = nc.declare_dram_parameter("wo", [HD, DIM], F32, isOutput=False)
    out_ext = nc.declare_dram_parameter("out", [RN, DIM], F32, isOutput=True)

    # alternate PSUM->SBUF evictions between DVE and ScalarE
    _cp = [0]

    def cp(out, in_):
        if _cp[0] % 2 == 0:
            nc.vector.tensor_copy(out, in_)
        else:
            nc.scalar.copy(out, in_)
        _cp[0] += 1

    def dma(out, in_):
        nc.sync.dma_start(out=out, in_=in_)

    with tile.TileContext(nc) as tc:
        # ---- DRAM bounce buffers: one AllReduce per head-pair ----
        dram = tc.alloc_tile_pool(name="dram", bufs=1, space="DRAM")
        ar_in = [dram.tile([2 * N, N], BF16, tag=f"ar_in{hp}", name=f"ar_in{hp}") for hp in range(4)]
        wu_in = dram.tile([128, 8], BF16, tag="wu_in", name="wu_in")
        wu_out = dram.tile([128, 8], BF16, tag="wu_out", name="wu_out", addr_space="Shared")
        ar_out = [
            dram.tile([2 * N, N], BF16, tag=f"ar_out{hp}", name=f"ar_out{hp}", addr_space="Shared")
            for hp in range(4)
        ]

        # ---- SBUF pools, stacked so releases stay LIFO ----
        consts = tc.alloc_tile_pool(name="consts", bufs=1)
        vP_pool = tc.alloc_tile_pool(name="vP", bufs=RP * 4)       # 32 x 2KB
        et_pool = tc.alloc_tile_pool(name="et", bufs=8)            # 8KB
        attn_pool = tc.alloc_tile_pool(name="attn", bufs=16)       # 16KB
        rz_pool = tc.alloc_tile_pool(name="rz", bufs=2)
        rzbc_pool = tc.alloc_tile_pool(name="rzbc", bufs=4)
        zt_pool = tc.alloc_tile_pool(name="zt", bufs=2)            # 8KB
        xT_pool = tc.alloc_tile_pool(name="xT", bufs=1)            # 32KB
        xrow_pool = tc.alloc_tile_pool(name="xrow", bufs=4)        # 16KB
        wstage = tc.alloc_tile_pool(name="wstage", bufs=2)         # 8KB

        # constants
        wq_sb = consts.tile([128, 2, HD], BF16, tag="wq")
        wk_sb = consts.tile([128, 2, HD], BF16, tag="wk")
        wv_sb = consts.tile([128, 2, HD], BF16, tag="wv")
        wo_sb = consts.tile([128, 4, DIM], BF16, tag="wo")
        idf = consts.tile([128, 128], F32, tag="idf")
        ones_bf = consts.tile([128, 128], BF16, tag="ones_bf")
        onesf = consts.tile([128, 128], F32, tag="onesf")

        xT = xT_pool.tile([128, 2, RN], BF16, tag="xT")

        # first x chunk prefetch (4 row-blocks of 128 per DMA)
        first_xr = xrow_pool.tile([128, 4, DIM], F32, tag="xr")
        nc.sync.dma_start(
            out=first_xr[:],
            in_=x_ext[0:512, :].rearrange("(j p) d -> p j d", p=128),
        )

        for weng, wext, wsb in ((nc.sync, wq_ext, wq_sb), (nc.scalar, wk_ext, wk_sb)):
            wf = wstage.tile([128, 2, HD], F32, tag="wf")
            weng.dma_start(
                out=wf[:], in_=wext[:, :].rearrange("(k p) n -> p k n", p=128)
            )
            nc.any.tensor_copy(wsb[:], wf[:])
        make_identity(nc, idf[:])
        nc.vector.memset(onesf[:], 1.0)
        nc.vector.tensor_copy(ones_bf[:], onesf[:])

        # warm up ncfw so the first real AllReduce skips the cold-start lag
        nc.sync.dma_start(out=wu_in[:, :], in_=ones_bf[:, 0:8])
        nc.gpsimd.collective_compute(
            "AllReduce",
            mybir.AluOpType.add,
            replica_groups=RG,
            ins=[wu_in[:, :].opt()],
            outs=[wu_out[:, :].opt()],
        )

        # ---- x load + fp32 PE transpose -> xT bf16 [dim(2x128), rn] ----
        xp_psum = tc.alloc_tile_pool(name="xp_psum", bufs=2, space="PSUM")
        for c4 in range(RN // N):
            if c4 == 0:
                xr = first_xr
            else:
                xr = xrow_pool.tile([128, 4, DIM], F32, tag="xr")
                eng = nc.scalar if (c4 % 2 == 1) else nc.sync
                eng.dma_start(
                    out=xr[:],
                    in_=x_ext[c4 * 512:(c4 + 1) * 512, :].rearrange(
                        "(j p) d -> p j d", p=128
                    ),
                )
            for kc in range(2):
                pt = xp_psum.tile([128, N], F32, tag="xp")
                for j in range(4):
                    nc.tensor.transpose(
                        pt[:, j * 128:(j + 1) * 128],
                        xr[:, j, kc * 128:(kc + 1) * 128],
                        idf[:],
                    )
                cp(xT[:, kc, c4 * N:(c4 + 1) * N], pt[:])
        xp_psum.release()

        # late weight staging (wv used ~2/3 in, wo in the last quarter)
        wvf = wstage.tile([128, 2, HD], F32, tag="wf")
        nc.sync.dma_start(
            out=wvf[:], in_=wv_ext[:, :].rearrange("(k p) n -> p k n", p=128)
        )
        nc.any.tensor_copy(wv_sb[:], wvf[:])
        wof = wstage.tile([128, 4, DIM], F32, tag="wf")
        nc.sync.dma_start(
            out=wof[:], in_=wo_ext[:, :].rearrange("(k p) n -> p k n", p=128)
        )
        nc.any.tensor_copy(wo_sb[:], wof[:])
        wstage.release()
        xrow_pool.release()

        work_psum = tc.alloc_tile_pool(name="work_psum", bufs=6, space="PSUM")
        z_psum = tc.alloc_tile_pool(name="z_psum", bufs=1, space="PSUM")

        stage_pool = tc.alloc_tile_pool(name="stage", bufs=2)      # 8KB
        qkP_pool = tc.alloc_tile_pool(name="qkP", bufs=40)         # 40KB

        attn = {}

        def softmax(hp, zpool, wait_ms):
            """exp + Z + 1/Z broadcast + normalize for AllReduce #hp."""
            with tc.tile_wait_until(wait_ms):
                for m in range(2):
                    h = 2 * hp + m
                    zt = zt_pool.tile([128, 4, N], BF16, tag="zt")
                    dma(
                        zt[:],
                        ar_out[hp][m * N:(m + 1) * N, :].rearrange(
                            "(jc p) n -> p jc n", p=128
                        ),
                    )
                    ets = []
                    for jc in range(4):
                        et = et_pool.tile([128, N], BF16, tag="et")
                        nc.scalar.activation(et[:], zt[:, jc, :], AF.Exp, scale=scale)
                        ets.append(et)
                    zp = zpool.tile([1, N], F32, tag="zp")
                    for jc in range(4):
                        nc.tensor.matmul(
                            zp[:],
                            ones_bf[:, 0:1],
                            ets[jc][:],
                            start=(jc == 0),
                            stop=(jc == 3),
                        )
                    lnz = rz_pool.tile([1, N], F32, tag="lnz")
                    nc.scalar.activation(lnz[:], zp[:], AF.Ln)
                    rz = rz_pool.tile([1, N], BF16, tag="rz")
                    with nc.allow_low_precision(reason="1/Z scale fine in bf16"):
                        nc.scalar.activation(rz[:], lnz[:], AF.Exp, scale=-1.0)
                    bp = zpool.tile([128, N], F32, tag="bp")
                    nc.tensor.matmul(
                        bp[:], ones_bf[0:1, :], rz[:], start=True, stop=True
                    )
                    rb = rzbc_pool.tile([128, N], BF16, tag="rzbc")
                    cp(rb[:], bp[:])
                    for jc in range(4):
                        at = attn_pool.tile([128, N], BF16, tag="attn")
                        nc.vector.tensor_mul(at[:], ets[jc][:], rb[:])
                        attn[(h, jc)] = at

        # ---- per head-pair: project q,k (row-pair packed), dots, AllReduce
        qP = {}
        kP = {}
        for hp in range(4):
            if hp == 3:
                softmax(0, z_psum, 0.105)
            for rr in range(R):
                rrp, rho = rr >> 1, rr & 1
                for wsb, pk in ((wq_sb, qP), (wk_sb, kP)):
                    ps = work_psum.tile([128, N], F32, tag="work")
                    for kc in range(2):
                        nc.tensor.matmul(
                            ps[:],
                            wsb[:, kc, hp * 128:(hp + 1) * 128],
                            xT[:, kc, rr * N:(rr + 1) * N],
                            start=(kc == 0),
                            stop=(kc == 1),
                        )
                    if rho == 0:
                        pk[(2 * hp, rrp)] = qkP_pool.tile([128, N], BF16, tag="pk", name="pk_e")
                        pk[(2 * hp + 1, rrp)] = qkP_pool.tile([128, N], BF16, tag="pk", name="pk_o")
                    cp(pk[(2 * hp, rrp)][rho * 64:(rho + 1) * 64, :], ps[0:64, :])
                    cp(pk[(2 * hp + 1, rrp)][rho * 64:(rho + 1) * 64, :], ps[64:128, :])

            # dots: K=128 over (row-pair, d); rrp-major across 4 jc banks
            for m in range(2):
                h = 2 * hp + m
                st = stage_pool.tile([128, 4, N], BF16, tag="dstage")
                dps = [work_psum.tile([128, N], F32, tag="work", name=f"dots{jj}") for jj in range(4)]
                for rrp in range(RP):
                    for jc in range(4):
                        nc.tensor.matmul(
                            dps[jc][:],
                            kP[(h, rrp)][:, jc * 128:(jc + 1) * 128],
                            qP[(h, rrp)][:],
                            start=(rrp == 0),
                            stop=(rrp == RP - 1),
                            skip_group_check=True,
                        )
                for jc in range(4):
                    cp(st[:, jc, :], dps[jc][:])
                dma(
                    ar_in[hp][m * N:(m + 1) * N, :].rearrange(
                        "(jc p) n -> p jc n", p=128
                    ),
                    st[:],
                )

            nc.gpsimd.collective_compute(
                "AllReduce",
                mybir.AluOpType.add,
                replica_groups=RG,
                ins=[ar_in[hp][:, :].opt()],
                outs=[ar_out[hp][:, :].opt()],
            )

        # ---- v projection (overlaps the AllReduces; reads xT) ----
        vP = {}
        for rr in range(R):
            rrp, rho = rr >> 1, rr & 1
            if rr == 2:
                softmax(1, z_psum, 0.135)
            if rr == 9:
                softmax(2, z_psum, 0.165)
            for jt in range(4):
                ps = work_psum.tile([128, N], F32, tag="work")
                for kc in range(2):
                    nc.tensor.matmul(
                        ps[:],
                        xT[:, kc, rr * N + jt * 128:rr * N + jt * 128 + 128],
                        wv_sb[:, kc, :],
                        start=(kc == 0),
                        stop=(kc == 1),
                    )
                if rho == 0:
                    vP[(rrp, jt)] = vP_pool.tile([128, H, 128], BF16, tag="vP", name="vPt")
                cp(
                    vP[(rrp, jt)][:, :, rho * 64:(rho + 1) * 64],
                    ps[:].rearrange("p (h d) -> p h d", d=64),
                )

        z_psum.release()
        work_psum.release()
        qkP_pool.release()
        stage_pool.release()
        xT_pool.release()

        # ---- attn^T @ v -> per-row oT, then out @ Wo ----
        oT_pool = tc.alloc_tile_pool(name="oT", bufs=R * 4)        # 64KB
        fst_pool = tc.alloc_tile_pool(name="fst", bufs=3)
        av_psum = tc.alloc_tile_pool(name="av_psum", bufs=4, space="PSUM")
        fin_psum = tc.alloc_tile_pool(name="fin_psum", bufs=2, space="PSUM")
        z2_psum = tc.alloc_tile_pool(name="z2_psum", bufs=1, space="PSUM")

        _oq = [0]
        oT = {}
        for hp in range(4):
            if hp == 0:
                softmax(3, z2_psum, 0.195)
            for rrp in range(RP):
                for m in range(2):
                    h = 2 * hp + m
                    ap_ = av_psum.tile([128, N], F32, tag="av")
                    for jt in range(4):
                        nc.tensor.matmul(
                            ap_[:],
                            vP[(rrp, jt)][:, h, :],
                            attn[(h, jt)][:],
                            start=(jt == 0),
                            stop=(jt == 3),
                        )
                    for rho in range(2):
                        r = 2 * rrp + rho
                        if (r, hp) not in oT:
                            oT[(r, hp)] = oT_pool.tile([128, N], BF16, tag="oT", name="oTt")
                        cp(
                            oT[(r, hp)][m * 64:(m + 1) * 64, :],
                            ap_[rho * 64:(rho + 1) * 64, :],
                        )
                if hp == 3:
                    # output projection for the two rows of this pair
                    for rho in range(2):
                        r = 2 * rrp + rho
                        fst = fst_pool.tile([128, 4, DIM], F32, tag="fst")
                        for icp in range(2):
                            psf = fin_psum.tile([128, 2, DIM], F32, tag="fin")
                            for ici in range(2):
                                ic = 2 * icp + ici
                                for kc in range(4):
                                    nc.tensor.matmul(
                                        psf[:, ici, :],
                                        oT[(r, kc)][:, ic * 128:(ic + 1) * 128],
                                        wo_sb[:, kc, :],
                                        start=(kc == 0),
                                        stop=(kc == 3),
                                    )
                            cp(fst[:, 2 * icp:2 * icp + 2, :], psf[:])
                        eng = nc.gpsimd if _oq[0] % 2 == 0 else nc.sync
                        eng.dma_start(
                            out=out_ext[r * N:(r + 1) * N, :].rearrange(
                                "(ic p) d -> p ic d", p=128
                            ),
                            in_=fst[:],
                        )
                        _oq[0] += 1

        z2_psum.release()
        fin_psum.release()
        av_psum.release()
        fst_pool.release()
        oT_pool.release()
        zt_pool.release()
        rzbc_pool.release()
        rz_pool.release()
        attn_pool.release()
        et_pool.release()
        vP_pool.release()
        consts.release()
        dram.release()

    if not nc.is_finalized():
        nc.finalize()
    return nc


_cache = {}


def _get_nc(scale: float):
    key = round(float(scale), 12)
    if key not in _cache:
        _cache[key] = build_nc(float(scale))
    return _cache[key]


def make_in_maps(x, Wq, Wkv, Wo):
    x = np.ascontiguousarray(np.asarray(x, dtype=np.float32)).reshape(CORES, RN, DIM)
    Wq = np.ascontiguousarray(np.asarray(Wq, dtype=np.float32))
    Wkv = np.asarray(Wkv, dtype=np.float32)
    Wk = np.ascontiguousarray(Wkv[:, :HD])
    Wv = np.ascontiguousarray(Wkv[:, HD:])
    Wo = np.ascontiguousarray(np.asarray(Wo, dtype=np.float32))
    return [
        {"x": x[c], "wq": Wq, "wk": Wk, "wv": Wv, "wo": Wo} for c in range(CORES)
    ]


def kernel(x, Wq, Wkv, Wo, bo, mask, tie_attn_dim):
    x = np.asarray(x)
    br, n, dim = x.shape
    r = int(tie_attn_dim)
    assert (br, n, dim) == (128, 512, 256) and r == 128, "kernel hardcodes shapes"
    mask = np.asarray(mask)
    assert mask.all(), "kernel assumes an all-valid mask"
    num_rows = float(mask.reshape(1, r, n).any(axis=-1).sum(axis=-1)[0])
    scale = (D ** -0.5) * (num_rows ** -0.5)

    nc = _get_nc(scale)
    in_maps = make_in_maps(x, Wq, Wkv, Wo)
    res = bass_utils.run_bass_kernel_spmd(nc, in_maps, core_ids=list(range(CORES)))
    out = np.concatenate([m["out"] for m in res.results], axis=0)
    out = out.reshape(br, n, dim)
    bo = np.asarray(bo, dtype=np.float32)
    if bo.any():
        out = out + bo
    return np.ascontiguousarray(out.astype(np.float32))


# revision 11
# speedup vs baseline: 1.1368x; 1.0264x over previous
"""Tied-row (MSA) attention, sharded over 8 TRN2 NeuronCores.

Reference computation (b=1, r=128 MSA rows, n=512, 8 heads x 64):
    q, k, v = x @ Wq, x @ Wk, x @ Wv          per-row projections
    dots[h,i,j] = sum_{r,d} q[r,h,i,d] k[r,h,j,d] * scale / sqrt(num_rows)
    attn = softmax_j(dots)                     shared across rows
    out[r,i] = (sum_j attn[h,i,j] v[r,h,j,d]) @ Wo + bo

Sharding: MSA-row axis r split 16-per-core; partial logits are AllReduced
(one bf16 AllReduce per head-pair, pipelined behind later pairs' compute).

Every matmul runs at full 128-wide PE contraction by packing MSA ROW-PAIRS
into the partition axis:
  - dots: qP/kP tiles hold (row-pair, head) data as [rho*64+d, token]; one
    K=128 matmul reduces two rows at once (the tied-row r-reduction makes
    the cross-row sum exactly what we want).  256 matmuls instead of 512.
  - attn@v: vP tiles [j, h, rho*64+d] give a [128,128] stationary per
    (head, row-pair); the row-tied attention tile is the shared moving
    side.  Output [(rho,d), i] psum halves route to per-row oT tiles
    (cross-partition-offset evictions).  256 matmuls instead of 512.
  - x transposes on the PE in fp32 (2 cyc/row); the fp32->bf16 cast rides
    the psum eviction, removing the DVE cast pass entirely.
  - softmax: Z[i] from a [128,1]-ones matmul, 1/Z via the ACT Reciprocal
    LUT on the [1,512] row, broadcast across partitions with a K=1 matmul.
"""

import numpy as np

import concourse.bacc as bacc
import concourse.bass as bass
import concourse.mybir as mybir
import concourse.tile as tile
from concourse import bass_utils
from concourse.masks import make_identity

CORES = 8
R = 16          # MSA rows per core
RP = R // 2     # row pairs per core
N = 512         # sequence length
DIM = 256       # model dim
H = 8           # heads
D = 64          # head dim
HD = H * D      # 512
RN = R * N      # 8192 token-rows per core

F32 = mybir.dt.float32
BF16 = mybir.dt.bfloat16
AF = mybir.ActivationFunctionType

RG = [list(range(CORES))]


def build_nc(scale: float):
    nc = bacc.Bacc(None, target_bir_lowering=False, debug=False)

    x_ext = nc.declare_dram_parameter("x", [RN, DIM], F32, isOutput=False)
    wq_ext = nc.declare_dram_parameter("wq", [DIM, HD], F32, isOutput=False)
    wk_ext = nc.declare_dram_parameter("wk", [DIM, HD], F32, isOutput=False)
    wv_ext = nc.declare_dram_parameter("wv", [DIM, HD], F32, isOutput=False)
    wo_ext = nc.declare_dram_parameter("wo", [HD, DIM], F32, isOutput=False)
    out_ext = nc.declare_dram_parameter("out", [RN, DIM], F32, isOutput=True)

    # alternate PSUM->SBUF evictions between DVE and ScalarE
    _cp = [0]

    def cp(out, in_):
        if _cp[0] % 2 == 0:
            nc.vector.tensor_copy(out, in_)
        else:
            nc.scalar.copy(out, in_)
        _cp[0] += 1

    def dma(out, in_):
        nc.sync.dma_start(out=out, in_=in_)

    with tile.TileContext(nc) as tc:
        # ---- DRAM bounce buffers: one AllReduce per head-pair ----
        dram = tc.alloc_tile_pool(name="dram", bufs=1, space="DRAM")
        ar_in = [dram.tile([2 * N, N], BF16, tag=f"ar_in{hp}", name=f"ar_in{hp}") for hp in range(4)]
        wu_in = dram.tile([128, 8], BF16, tag="wu_in", name="wu_in")
        wu_out = dram.tile([128, 8], BF16, tag="wu_out", name="wu_out", addr_space="Shared")
        ar_out = [
            dram.tile([2 * N, N], BF16, tag=f"ar_out{hp}", name=f"ar_out{hp}", addr_space="Shared")
            for hp in range(4)
        ]

        # ---- SBUF pools, stacked so releases stay LIFO ----
        consts = tc.alloc_tile_pool(name="consts", bufs=1)
        vP_pool = tc.alloc_tile_pool(name="vP", bufs=RP * 4)       # 32 x 2KB
        et_pool = tc.alloc_tile_pool(name="et", bufs=8)            # 8KB
        attn_pool = tc.alloc_tile_pool(name="attn", bufs=16)       # 16KB
        rz_pool = tc.alloc_tile_pool(name="rz", bufs=2)
        rzbc_pool = tc.alloc_tile_pool(name="rzbc", bufs=4)
        zt_pool = tc.alloc_tile_pool(name="zt", bufs=2)            # 8KB
        xT_pool = tc.alloc_tile_pool(name="xT", bufs=1)            # 32KB
        xrow_pool = tc.alloc_tile_pool(name="xrow", bufs=4)        # 16KB
        wstage = tc.alloc_tile_pool(name="wstage", bufs=2)         # 8KB

        # constants
        wq_sb = consts.tile([128, 2, HD], BF16, tag="wq")
        wk_sb = consts.tile([128, 2, HD], BF16, tag="wk")
        wv_sb = consts.tile([128, 2, HD], BF16, tag="wv")
        wo_sb = consts.tile([128, 4, DIM], BF16, tag="wo")
        idf = consts.tile([128, 128], F32, tag="idf")
        ones_bf = consts.tile([128, 128], BF16, tag="ones_bf")
        onesf = consts.tile([128, 128], F32, tag="onesf")

        xT = xT_pool.tile([128, 2, RN], BF16, tag="xT")

        # first x chunk prefetch (4 row-blocks of 128 per DMA)
        first_xr = xrow_pool.tile([128, 4, DIM], F32, tag="xr")
        for half, eng in ((0, nc.sync), (1, nc.scalar)):
            eng.dma_start(
                out=first_xr[:, 2 * half:2 * half + 2, :],
                in_=x_ext[half * 256:half * 256 + 256, :].rearrange(
                    "(j p) d -> p j d", p=128
                ),
            )

        for weng, wext, wsb in ((nc.sync, wq_ext, wq_sb), (nc.scalar, wk_ext, wk_sb)):
            wf = wstage.tile([128, 2, HD], F32, tag="wf")
            weng.dma_start(
                out=wf[:], in_=wext[:, :].rearrange("(k p) n -> p k n", p=128)
            )
            nc.any.tensor_copy(wsb[:], wf[:])
        make_identity(nc, idf[:])
        nc.vector.memset(onesf[:], 1.0)
        nc.vector.tensor_copy(ones_bf[:], onesf[:])

        # warm up ncfw so the first real AllReduce skips the cold-start lag
        nc.sync.dma_start(out=wu_in[:, :], in_=ones_bf[:, 0:8])
        nc.gpsimd.collective_compute(
            "AllReduce",
            mybir.AluOpType.add,
            replica_groups=RG,
            ins=[wu_in[:, :].opt()],
            outs=[wu_out[:, :].opt()],
        )

        # ---- x load + fp32 PE transpose -> xT bf16 [dim(2x128), rn] ----
        xp_psum = tc.alloc_tile_pool(name="xp_psum", bufs=2, space="PSUM")
        for c4 in range(RN // N):
            if c4 == 0:
                xr = first_xr
            else:
                xr = xrow_pool.tile([128, 4, DIM], F32, tag="xr")
                for half, eng in ((0, nc.sync), (1, nc.scalar)):
                    eng.dma_start(
                        out=xr[:, 2 * half:2 * half + 2, :],
                        in_=x_ext[c4 * 512 + half * 256:c4 * 512 + half * 256 + 256, :].rearrange(
                            "(j p) d -> p j d", p=128
                        ),
                    )
            for kc in range(2):
                pt = xp_psum.tile([128, N], F32, tag="xp")
                for j in range(4):
                    nc.tensor.transpose(
                        pt[:, j * 128:(j + 1) * 128],
                        xr[:, j, kc * 128:(kc + 1) * 128],
                        idf[:],
                    )
                cp(xT[:, kc, c4 * N:(c4 + 1) * N], pt[:])
        xp_psum.release()

        # late weight staging (wv used ~2/3 in, wo in the last quarter)
        wvf = wstage.tile([128, 2, HD], F32, tag="wf")
        nc.sync.dma_start(
            out=wvf[:], in_=wv_ext[:, :].rearrange("(k p) n -> p k n", p=128)
        )
        nc.any.tensor_copy(wv_sb[:], wvf[:])
        wof = wstage.tile([128, 4, DIM], F32, tag="wf")
        nc.sync.dma_start(
            out=wof[:], in_=wo_ext[:, :].rearrange("(k p) n -> p k n", p=128)
        )
        nc.any.tensor_copy(wo_sb[:], wof[:])
        wstage.release()
        xrow_pool.release()

        work_psum = tc.alloc_tile_pool(name="work_psum", bufs=6, space="PSUM")
        z_psum = tc.alloc_tile_pool(name="z_psum", bufs=1, space="PSUM")

        stage_pool = tc.alloc_tile_pool(name="stage", bufs=2)      # 8KB
        qkP_pool = tc.alloc_tile_pool(name="qkP", bufs=40)         # 40KB

        attn = {}

        def softmax(hp, zpool, wait_ms):
            """exp + Z + 1/Z broadcast + normalize for AllReduce #hp.
            ACT LUT switches are batched: Exp x8, Ln x2, Exp(-x) x2 -> two
            table reloads per pair instead of per head."""
            with tc.tile_wait_until(wait_ms):
                ets = {}
                zps = []
                for m in range(2):
                    h = 2 * hp + m
                    zt = zt_pool.tile([128, 4, N], BF16, tag="zt")
                    dma(
                        zt[:],
                        ar_out[hp][m * N:(m + 1) * N, :].rearrange(
                            "(jc p) n -> p jc n", p=128
                        ),
                    )
                    for jc in range(4):
                        et = et_pool.tile([128, N], BF16, tag="et")
                        nc.scalar.activation(et[:], zt[:, jc, :], AF.Exp, scale=scale)
                        ets[(m, jc)] = et
                zp = zpool.tile([65, N], F32, tag="zp")
                for m in range(2):
                    for jc in range(4):
                        nc.tensor.matmul(
                            zp[64 * m:64 * m + 1, :],
                            ones_bf[:, 0:1],
                            ets[(m, jc)][:],
                            start=(jc == 0),
                            stop=(jc == 3),
                            skip_group_check=True,
                        )
                lnzs = [rz_pool.tile([1, N], F32, tag="lnz", name=f"lnz{m}") for m in range(2)]
                for m in range(2):
                    nc.scalar.activation(lnzs[m][:], zp[64 * m:64 * m + 1, :], AF.Ln)
                rzs = [rz_pool.tile([1, N], BF16, tag="rz", name=f"rz{m}") for m in range(2)]
                with nc.allow_low_precision(reason="1/Z scale fine in bf16"):
                    for m in range(2):
                        nc.scalar.activation(rzs[m][:], lnzs[m][:], AF.Exp, scale=-1.0)
                for m in range(2):
                    h = 2 * hp + m
                    bp = zpool.tile([128, N], F32, tag="bp")
                    nc.tensor.matmul(
                        bp[:], ones_bf[0:1, :], rzs[m][:], start=True, stop=True
                    )
                    rb = rzbc_pool.tile([128, N], BF16, tag="rzbc")
                    cp(rb[:], bp[:])
                    for jc in range(4):
                        at = attn_pool.tile([128, N], BF16, tag="attn")
                        nc.vector.tensor_mul(at[:], ets[(m, jc)][:], rb[:])
                        attn[(h, jc)] = at

        # ---- per head-pair: project q,k (row-pair packed), dots, AllReduce
        qP = {}
        kP = {}
        for hp in range(4):
            if hp == 3:
                softmax(0, z_psum, 0.105)
            for rr in range(R):
                rrp, rho = rr >> 1, rr & 1
                for wsb, pk in ((wq_sb, qP), (wk_sb, kP)):
                    ps = work_psum.tile([128, N], F32, tag="work")
                    for kc in range(2):
                        nc.tensor.matmul(
                            ps[:],
                            wsb[:, kc, hp * 128:(hp + 1) * 128],
                            xT[:, kc, rr * N:(rr + 1) * N],
                            start=(kc == 0),
                            stop=(kc == 1),
                        )
                    if rho == 0:
                        pk[(2 * hp, rrp)] = qkP_pool.tile([128, N], BF16, tag="pk", name="pk_e")
                        pk[(2 * hp + 1, rrp)] = qkP_pool.tile([128, N], BF16, tag="pk", name="pk_o")
                    cp(pk[(2 * hp, rrp)][rho * 64:(rho + 1) * 64, :], ps[0:64, :])
                    cp(pk[(2 * hp + 1, rrp)][rho * 64:(rho + 1) * 64, :], ps[64:128, :])

            # dots: K=128 over (row-pair, d); rrp-major across 4 jc banks
            for m in range(2):
                h = 2 * hp + m
                st = stage_pool.tile([128, 4, N], BF16, tag="dstage")
                dps = [work_psum.tile([128, N], F32, tag="work", name=f"dots{jj}") for jj in range(4)]
                for rrp in range(RP):
                    for jc in range(4):
                        nc.tensor.matmul(
                            dps[jc][:],
                            kP[(h, rrp)][:, jc * 128:(jc + 1) * 128],
                            qP[(h, rrp)][:],
                            start=(rrp == 0),
                            stop=(rrp == RP - 1),
                            skip_group_check=True,
                        )
                for jc in range(4):
                    cp(st[:, jc, :], dps[jc][:])
                dma(
                    ar_in[hp][m * N:(m + 1) * N, :].rearrange(
                        "(jc p) n -> p jc n", p=128
                    ),
                    st[:],
                )

            nc.gpsimd.collective_compute(
                "AllReduce",
                mybir.AluOpType.add,
                replica_groups=RG,
                ins=[ar_in[hp][:, :].opt()],
                outs=[ar_out[hp][:, :].opt()],
            )

        # ---- v projection (overlaps the AllReduces; reads xT) ----
        vP = {}
        for rr in range(R):
            rrp, rho = rr >> 1, rr & 1
            if rr == 2:
                softmax(1, z_psum, 0.135)
            if rr == 9:
                softmax(2, z_psum, 0.165)
            for jt in range(4):
                ps = work_psum.tile([128, N], F32, tag="work")
                for kc in range(2):
                    nc.tensor.matmul(
                        ps[:],
                        xT[:, kc, rr * N + jt * 128:rr * N + jt * 128 + 128],
                        wv_sb[:, kc, :],
                        start=(kc == 0),
                        stop=(kc == 1),
                    )
                if rho == 0:
                    vP[(rrp, jt)] = vP_pool.tile([128, H, 128], BF16, tag="vP", name="vPt")
                cp(
                    vP[(rrp, jt)][:, :, rho * 64:(rho + 1) * 64],
                    ps[:].rearrange("p (h d) -> p h d", d=64),
                )

        z_psum.release()
        work_psum.release()
        qkP_pool.release()
        stage_pool.release()
        xT_pool.release()

        # ---- attn^T @ v -> per-row oT, then out @ Wo ----
        oT_pool = tc.alloc_tile_pool(name="oT", bufs=R * 4)        # 64KB
        fst_pool = tc.alloc_tile_pool(name="fst", bufs=3)
        av_psum = tc.alloc_tile_pool(name="av_psum", bufs=4, space="PSUM")
        fin_psum = tc.alloc_tile_pool(name="fin_psum", bufs=2, space="PSUM")
        z2_psum = tc.alloc_tile_pool(name="z2_psum", bufs=1, space="PSUM")

        _oq = [0]
        oT = {}
        for hp in range(4):
            if hp == 0:
                softmax(3, z2_psum, 0.195)
            for rrp in range(RP):
                for m in range(2):
                    h = 2 * hp + m
                    ap_ = av_psum.tile([128, N], F32, tag="av")
                    for jt in range(4):
                        nc.tensor.matmul(
                            ap_[:],
                            vP[(rrp, jt)][:, h, :],
                            attn[(h, jt)][:],
                            start=(jt == 0),
                            stop=(jt == 3),
                        )
                    for rho in range(2):
                        r = 2 * rrp + rho
                        if (r, hp) not in oT:
                            oT[(r, hp)] = oT_pool.tile([128, N], BF16, tag="oT", name="oTt")
                        cp(
                            oT[(r, hp)][m * 64:(m + 1) * 64, :],
                            ap_[rho * 64:(rho + 1) * 64, :],
                        )
                if hp == 3:
                    # output projection for the two rows of this pair
                    for rho in range(2):
                        r = 2 * rrp + rho
                        fst = fst_pool.tile([128, 4, DIM], F32, tag="fst")
                        for icp in range(2):
                            psf = fin_psum.tile([128, 2, DIM], F32, tag="fin")
                            for ici in range(2):
                                ic = 2 * icp + ici
                                for kc in range(4):
                                    nc.tensor.matmul(
                                        psf[:, ici, :],
                                        oT[(r, kc)][:, ic * 128:(ic + 1) * 128],
                                        wo_sb[:, kc, :],
                                        start=(kc == 0),
                                        stop=(kc == 3),
                                    )
                            cp(fst[:, 2 * icp:2 * icp + 2, :], psf[:])
                        eng = nc.gpsimd if _oq[0] % 2 == 0 else nc.sync
                        eng.dma_start(
                            out=out_ext[r * N:(r + 1) * N, :].rearrange(
                                "(ic p) d -> p ic d", p=128
                            ),
                            in_=fst[:],
                        )
                        _oq[0] += 1

        z2_psum.release()
        fin_psum.release()
        av_psum.release()
        fst_pool.release()
        oT_pool.release()
        zt_pool.release()
        rzbc_pool.release()
        rz_pool.release()
        attn_pool.release()
        et_pool.release()
        vP_pool.release()
        consts.release()
        dram.release()

    if not nc.is_finalized():
        nc.finalize()
    return nc


_cache = {}


def _get_nc(scale: float):
    key = round(float(scale), 12)
    if key not in _cache:
        _cache[key] = build_nc(float(scale))
    return _cache[key]


def make_in_maps(x, Wq, Wkv, Wo):
    x = np.ascontiguousarray(np.asarray(x, dtype=np.float32)).reshape(CORES, RN, DIM)
    Wq = np.ascontiguousarray(np.asarray(Wq, dtype=np.float32))
    Wkv = np.asarray(Wkv, dtype=np.float32)
    Wk = np.ascontiguousarray(Wkv[:, :HD])
    Wv = np.ascontiguousarray(Wkv[:, HD:])
    Wo = np.ascontiguousarray(np.asarray(Wo, dtype=np.float32))
    return [
        {"x": x[c], "wq": Wq, "wk": Wk, "wv": Wv, "wo": Wo} for c in range(CORES)
    ]


def kernel(x, Wq, Wkv, Wo, bo, mask, tie_attn_dim):
    x = np.asarray(x)
    br, n, dim = x.shape
    r = int(tie_attn_dim)
    assert (br, n, dim) == (128, 512, 256) and r == 128, "kernel hardcodes shapes"
    mask = np.asarray(mask)
    assert mask.all(), "kernel assumes an all-valid mask"
    num_rows = float(mask.reshape(1, r, n).any(axis=-1).sum(axis=-1)[0])
    scale = (D ** -0.5) * (num_rows ** -0.5)

    nc = _get_nc(scale)
    in_maps = make_in_maps(x, Wq, Wkv, Wo)
    res = bass_utils.run_bass_kernel_spmd(nc, in_maps, core_ids=list(range(CORES)))
    out = np.concatenate([m["out"] for m in res.results], axis=0)
    out = out.reshape(br, n, dim)
    bo = np.asarray(bo, dtype=np.float32)
    if bo.any():
        out = out + bo
    return np.ascontiguousarray(out.astype(np.float32))
